# revision 1
# baseline (speedup 1.0000x reference)
"""CharEmb kernel for Trainium2 (8 NeuronCores, batch-sharded).

Computation (per word of 32 chars):
  emb = table[ids]                  # [32 chars, 64] gathered fp32
  x[i, j] = emb[i//2, 32*(i%2)+j]   # raw-buffer reshape [64, 32]
  y[f, t] = sum_{i,k} x[i, t+k] * w[f, i, k]   (valid conv, K=3)
  out[f] = max_t y[f, t] + b[f]

Device mapping per core (2048 words = 65536 chars):
  - dma_gather: char c -> partition c%128, its 64-fp32 table row on the
    free dim.  A 128-partition block = 4 words (slot s = partitions
    32s..32s+32 = word 4b+s of block b).
  - conv: contraction over (p, h, k) = (char-in-word, half, tap) as 6
    accumulating K=32 matmuls per word-slot, row-tiled across the 4
    slots via tile_position.  rhs column j0=32h+k..+30 of the gathered
    rows; stationary W[h,k][p, f] = conv_w[f, 2p+h, k].
  - maxpool: per-word tensor_reduce(max) over the 30 t columns in PSUM.
"""

import sys
from contextlib import ExitStack

import numpy as np

if "/opt/trn_rl_repo" not in sys.path:
    sys.path.insert(0, "/opt/trn_rl_repo")

import concourse.bass as bass
import concourse.tile as tile
from concourse import bacc, mybir
from concourse.bass_utils import run_bass_kernel_spmd

# Problem constants (hardcoded per spec)
B, S, C = 32, 512, 32
V, E = 101, 64
F, K = 128, 3
T = C - K + 1  # 30 valid conv positions
NCORES = 8
WORDS = (B * S) // NCORES  # 2048 words per core
NCHARS = WORDS * C  # 65536

CHUNK_WORDS = 64  # words per pipeline chunk
NCHUNKS = WORDS // CHUNK_WORDS  # 32
BLOCKS = CHUNK_WORDS // 4  # 16 gather blocks (128 chars) per chunk
CHUNK_IDX_COLS = (CHUNK_WORDS * C) // 16  # 128 idx columns per chunk

f32 = mybir.dt.float32
f32r = mybir.dt.float32r
bf16 = mybir.dt.bfloat16
i16 = mybir.dt.int16


def build_kernel(words=WORDS, chunk_words=CHUNK_WORDS, num_devices=NCORES,
                 debug_obuf=False, add_bias=True):
    nchunks = words // chunk_words
    blocks = chunk_words // 4
    idx_cols_per_chunk = (chunk_words * C) // 16

    nc = bacc.Bacc(
        "TRN2",
        target_bir_lowering=False,
        debug=False,
        enable_asserts=True,
        num_devices=num_devices,
    )

    idx_d = nc.dram_tensor("idx", [128, (words * C) // 16], i16, kind="ExternalInput")
    tab_d = nc.dram_tensor("tab", [V, E], f32r, kind="ExternalInput")
    w_d = nc.dram_tensor("wmat", [128, 6 * 128], f32r, kind="ExternalInput")
    b_d = nc.dram_tensor("bias", [128, 1], f32, kind="ExternalInput")
    # f-major output: out[f, col] with col = 64c + 16s + w -> word 64c + 4w + s
    out_d = nc.dram_tensor("out", [128, words], f32, kind="ExternalOutput")
    if debug_obuf:
        dbg_d = nc.dram_tensor("dbg_obuf", [128, words], f32, kind="ExternalOutput")

    with tile.TileContext(nc) as tc, ExitStack() as ctx:
        const_pool = ctx.enter_context(tc.tile_pool(name="const", bufs=1))
        g_pool = ctx.enter_context(tc.tile_pool(name="gath", bufs=3))
        p_pool = ctx.enter_context(tc.tile_pool(name="psum", bufs=2, space="PSUM"))

        idx_sb = const_pool.tile([128, (words * C) // 16], i16)
        w_sb = const_pool.tile([128, 6 * 128], f32r)
        b_sb = const_pool.tile([128, 1], f32)
        obuf = const_pool.tile([128, words], f32)

        nc.sync.dma_start(idx_sb[:], idx_d.ap())
        nc.sync.dma_start(w_sb[:], w_d.ap())
        nc.sync.dma_start(b_sb[:], b_d.ap())

        for c in range(nchunks):
            # --- gather embeddings for this chunk's 2048 chars ---
            g = g_pool.tile([128, blocks * E], f32r)
            g_r = g[:].rearrange("p (b e) -> p b e", e=E)
            nc.gpsimd.dma_gather(
                out_ap=g_r,
                in_ap=tab_d.ap(),
                idxs_ap=idx_sb[:, c * idx_cols_per_chunk:(c + 1) * idx_cols_per_chunk],
                num_idxs=chunk_words * C,
                num_idxs_reg=chunk_words * C,
                elem_size=E,
                single_packet=False,
            )

            # --- conv: 6 accumulating matmuls x 4 row-tiled slots ---
            p = p_pool.tile([128, 4 * 512], f32)
            for hk in range(6):
                h, k = divmod(hk, 3)
                j0 = 32 * h + k
                for s in range(4):
                    out_ap = (
                        p[:, 512 * s:512 * s + blocks * T]
                        .rearrange("f (w t) -> f w t", t=T)
                    )
                    rhs = g_r[32 * s:32 * s + 32, :, j0:j0 + T]
                    lhsT = w_sb[32 * s:32 * s + 32, 128 * hk:128 * hk + 128]
                    nc.tensor.matmul(
                        out_ap,
                        lhsT,
                        rhs,
                        start=(hk == 0),
                        stop=(hk == 5),
                        tile_position=(32 * s, 0),
                        skip_group_check=True,
                    )

            # --- maxpool over t (per word) ---
            p_v = (
                p[:].rearrange("f (s x) -> f s x", x=512)[:, :, 0:blocks * T]
                .rearrange("f s (w t) -> f s w t", t=T)
            )
            o_v = (
                obuf[:, c * chunk_words:(c + 1) * chunk_words]
                .rearrange("f (s w) -> f s w", w=blocks)
            )
            nc.vector.tensor_reduce(
                o_v, p_v, axis=mybir.AxisListType.X, op=mybir.AluOpType.max
            )

        # --- bias + store ---
        if debug_obuf:
            nc.sync.dma_start(dbg_d.ap(), obuf[:])
        if add_bias:
            nc.vector.tensor_scalar_add(obuf[:], obuf[:], b_sb[:, 0:1])
        nc.sync.dma_start(out_d.ap(), obuf[:])

    nc.compile()
    return nc


def host_prep(char_ids, emb_table, conv_w, conv_b, words=WORDS, num_devices=NCORES):
    """Build per-core input maps from full inputs."""
    char_ids = np.asarray(char_ids)
    emb_table = np.ascontiguousarray(np.asarray(emb_table), dtype=np.float32)
    conv_w = np.asarray(conv_w, dtype=np.float32)
    conv_b = np.asarray(conv_b, dtype=np.float32)

    ids_flat = char_ids.reshape(-1, C).astype(np.int16)  # [16384, 32]

    # stationary weights: wmat[32s+p, 128*(3h+k) + f] = conv_w[f, 2p+h, k]
    wmat = np.zeros((128, 6 * 128), dtype=np.float32)
    for h in range(2):
        for k in range(3):
            hk = 3 * h + k
            w_pf = conv_w[:, h::2, k].T  # [32 p, 128 f]
            wmat[:, 128 * hk:128 * (hk + 1)] = np.tile(w_pf, (4, 1))

    bias = conv_b.reshape(128, 1)

    in_maps = []
    for j in range(num_devices):
        ids_core = ids_flat[j * words:(j + 1) * words]  # [words, 32]
        flat = ids_core.reshape(-1)  # char-major
        # wrap: char i -> [i%16, i//16], chunk-local columns
        ncols = flat.size // 16
        wrapped = flat.reshape(ncols, 16).T.copy()  # [16, ncols]
        idx = np.tile(wrapped, (8, 1))  # replicate to 128 partitions
        in_maps.append(
            {
                "idx": np.ascontiguousarray(idx),
                "tab": emb_table,
                "wmat": wmat,
                "bias": bias,
            }
        )
    return in_maps


def _ensure_ntff_hook():
    """The agent image's antenv lacks axon_hooks; shim it and install the
    ctypes NTFF profiling hook so trace=True yields HW exec times."""
    import types

    if "antenv.axon_hooks" in sys.modules:
        return
    mod = types.ModuleType("antenv.axon_hooks")
    _hook = [None]
    mod.get_axon_ntff_profile_hook = lambda: _hook[0]
    mod.set_axon_ntff_profile_hook = lambda h: _hook.__setitem__(0, h)
    sys.modules["antenv.axon_hooks"] = mod
    try:
        import antenv

        antenv.axon_hooks = mod
        from trn_agent_boot.trn_boot import _ntff_profile_via_ctypes

        hook = _ntff_profile_via_ctypes("/opt/axon/libaxon_pjrt.so")
        mod.set_axon_ntff_profile_hook(hook)
    except Exception as e:  # degrade to no-trace
        print(f"ntff hook install failed: {e}", file=sys.stderr)


_NC_CACHE = {}


def _get_nc():
    if "nc" not in _NC_CACHE:
        _NC_CACHE["nc"] = build_kernel()
    return _NC_CACHE["nc"]


def unscramble_out(raw, words=WORDS, chunk_words=CHUNK_WORDS):
    """[128 f, words] f-major, col = 64c+16s+w  ->  [words, 128] word-major."""
    blocks = chunk_words // 4
    nchunks = words // chunk_words
    o = raw.reshape(128, nchunks, 4, blocks)  # [f, c, s, w]
    o = o.transpose(1, 3, 2, 0)  # [c, w, s, f]; word = 64c + 4w + s
    return np.ascontiguousarray(o.reshape(words, 128))


def kernel(char_ids, emb_table, conv_w, conv_b, trace=False):
    if trace:
        _ensure_ntff_hook()
    nc = _get_nc()
    in_maps = host_prep(char_ids, emb_table, conv_w, conv_b)
    res = run_bass_kernel_spmd(
        nc, in_maps, core_ids=list(range(NCORES)), trace=trace
    )
    outs = [unscramble_out(res.results[j]["out"]) for j in range(NCORES)]
    full = np.concatenate(outs, axis=0).reshape(B, S, F).astype(np.float32)
    if trace:
        return full, res
    return full



# revision 15
# speedup vs baseline: 1.8593x; 1.8593x over previous
"""CharEmb kernel for Trainium2 (8 NeuronCores, batch-sharded).

Computation (per word of 32 chars):
  emb = table[ids]                  # [32 chars, 64] per word
  x[i, j] = emb[i//2, 32*(i%2)+j]   # raw-buffer reshape [64, 32]
  y[f, t] = sum_{i,k} x[i, t+k] * w[f, i, k]   (valid conv, K=3)
  out[f] = max_t y[f, t] + b[f]

The baseline's GPSIMD dma_gather (8ns/descriptor, 526us) is replaced by
a tensor-engine one-hot gather. Per chunk of 32 words (1024 chars):
  1. DMA broadcast: permuted ids -> SBUF [128, 1024] int16 (0-stride src).
     Host permutation: gather column 64*(w//2) + 2*p + (w%2) holds char
     (w, p), so the packed transpose below lands in conv-friendly form.
  2. DVE: one-hot via tensor_scalar(is_equal) vs partition iota (4x mode).
  3. PE: gather matmul, table bf16 [101, 64] stationary, one-hot rhs
     -> PSUM [64 e, 1024] fp32.
  4. Act: PSUM -> SBUF bf16 copy.
  5. DVE: 32x32 stream transpose on an int32-packed view (halves cost):
     T[32h+p, 64*(w//2) + 2j + w%2] = emb_w[p, 32h+j].
  6. DMA: shift-dup rows 64-127 = rows 0-63 shifted +2 bf16 cols (tap 1).
  7. PE conv per 16-word range: fused taps k=0,1 (128-row contraction)
     + tap k=2 (64 rows); t-window = strided column views.
  8. maxpool over t: DVE tensor_reduce on even chunks, Pool (gpsimd)
     pairwise max-tree on odd chunks.
Finally bias add + store out[f, word] (word-linear).
"""

import sys
from contextlib import ExitStack

import numpy as np

if "/opt/trn_rl_repo" not in sys.path:
    sys.path.insert(0, "/opt/trn_rl_repo")

import concourse.bass as bass
import concourse.tile as tile
from concourse import bacc, mybir
from concourse.bass_utils import run_bass_kernel_spmd

# Problem constants (hardcoded per spec)
B, S, C = 32, 512, 32
V, E = 101, 64
F, K = 128, 3
T = C - K + 1  # 30 valid conv positions
NCORES = 8
WORDS = (B * S) // NCORES  # 2048 words per core
NCHARS = WORDS * C  # 65536

CHUNK_W = 32                 # words per chunk
CH_COLS = CHUNK_W * C        # 1024 chars per chunk
NCHUNKS = WORDS // CHUNK_W   # 64
GROUP = 4                    # chunks per ids-broadcast DMA
RANGE_W = 16                 # words per conv matmul range
NRANGES = CHUNK_W // RANGE_W # 2

f32 = mybir.dt.float32
bf16 = mybir.dt.bfloat16
i16 = mybir.dt.int16
i32 = mybir.dt.int32


def build_kernel(num_devices=NCORES):
    nc = bacc.Bacc(
        "TRN2",
        target_bir_lowering=False,
        debug=False,
        enable_asserts=True,
        num_devices=num_devices,
    )

    idx_d = nc.dram_tensor("idx", [1, NCHARS], i16, kind="ExternalInput")
    tab_d = nc.dram_tensor("tab", [V, E], bf16, kind="ExternalInput")
    w_d = nc.dram_tensor("wmat", [128, 256], bf16, kind="ExternalInput")
    b_d = nc.dram_tensor("bias", [128, 1], f32, kind="ExternalInput")
    out_d = nc.dram_tensor("out", [128, WORDS], f32, kind="ExternalOutput")

    with tile.TileContext(nc) as tc, ExitStack() as ctx:
        const_pool = ctx.enter_context(tc.tile_pool(name="const", bufs=1))
        ids_pool = ctx.enter_context(tc.tile_pool(name="ids", bufs=3))
        oh_pool = ctx.enter_context(tc.tile_pool(name="oh", bufs=3))
        gsb_pool = ctx.enter_context(tc.tile_pool(name="gsb", bufs=3))
        t_pool = ctx.enter_context(tc.tile_pool(name="tt", bufs=3))
        ysb_pool = ctx.enter_context(tc.tile_pool(name="ysb", bufs=2))
        g_psum = ctx.enter_context(tc.tile_pool(name="gps", bufs=2, space="PSUM"))
        y_psum = ctx.enter_context(tc.tile_pool(name="yps", bufs=2, space="PSUM"))

        tab_sb = const_pool.tile([V, E], bf16)
        w_sb = const_pool.tile([128, 256], bf16)
        b_sb = const_pool.tile([128, 1], f32)
        iota_sb = const_pool.tile([128, 1], f32)
        obuf = const_pool.tile([128, WORDS], f32)

        nc.sync.dma_start(tab_sb[:], tab_d.ap())
        nc.sync.dma_start(w_sb[:], w_d.ap())
        nc.sync.dma_start(b_sb[:], b_d.ap())
        nc.gpsimd.iota(
            iota_sb[:], pattern=[[1, 1]], base=0, channel_multiplier=1,
            allow_small_or_imprecise_dtypes=True,
        )

        ids_t = None
        for cc in range(NCHUNKS):
            gidx, j = divmod(cc, GROUP)
            if j == 0:
                ids_t = ids_pool.tile([128, GROUP * CH_COLS], i16)
                src = idx_d.ap()[0:1, gidx * GROUP * CH_COLS:(gidx + 1) * GROUP * CH_COLS]
                nc.sync.dma_start(ids_t[:], src.broadcast_to((128, GROUP * CH_COLS)))

            # one-hot [101, 1024] bf16 (DVE 4x mode)
            oh = oh_pool.tile([128, CH_COLS], bf16)
            nc.vector.tensor_scalar(
                oh[0:V, :],
                ids_t[0:V, j * CH_COLS:(j + 1) * CH_COLS],
                iota_sb[0:V, 0:1],
                None,
                op0=mybir.AluOpType.is_equal,
            )

            # gather matmul -> PSUM [64, 1024] fp32
            g_ps = g_psum.tile([128, CH_COLS], f32)
            for half in range(2):
                nc.tensor.matmul(
                    g_ps[0:E, 512 * half:512 * (half + 1)],
                    tab_sb[0:V, 0:E],
                    oh[0:V, 512 * half:512 * (half + 1)],
                    start=True,
                    stop=True,
                )

            # PSUM -> SBUF bf16
            gsb = gsb_pool.tile([128, CH_COLS], bf16)
            nc.scalar.copy(gsb[0:E, :], g_ps[0:E, :])

            # packed 32x32 stream transpose (int32 view: pairs of bf16)
            t_t = t_pool.tile([128, CH_COLS], bf16)
            nc.vector.transpose(
                t_t[0:E, :].bitcast(i32), gsb[0:E, :].bitcast(i32)
            )

            # shift-dup: rows 64-127 = rows 0-63 shifted +2 bf16 cols (tap 1)
            nc.sync.dma_start(t_t[64:128, 0:CH_COLS - 2], t_t[0:64, 2:CH_COLS])

            # conv: fused taps (k=0 rows 0-63, k=1 dup rows) + tap k=2
            # T addr(w, j) = 64*(w//2) + 2*j + (w%2): dims (u, j, eps)
            # y PSUM: 2 banks, range r bank-aligned at col 512*r (480 used)
            y_ps = y_psum.tile([128, 2 * 512], f32)
            # rhs iteration (u, eps, t): addr = 64u + eps + 2j
            t_full = t_t[:, :].rearrange("q (u j e) -> q u e j", j=C, e=2)
            t_low = t_t[0:64, :].rearrange("q (u j e) -> q u e j", j=C, e=2)
            UR = RANGE_W // 2  # u per range (8)
            for r in range(NRANGES):
                # y col layout within range: 60*u' + 30*eps + t
                out_ap = (
                    y_ps[:, 512 * r:512 * r + UR * 2 * T]
                    .rearrange("f (u e t) -> f u e t", t=T, e=2)
                )
                nc.tensor.matmul(
                    out_ap,
                    w_sb[:, 0:128],
                    t_full[:, UR * r:UR * (r + 1), :, 0:T],
                    start=True,
                    stop=False,
                )
                nc.tensor.matmul(
                    out_ap,
                    w_sb[0:64, 128:256],
                    t_low[:, UR * r:UR * (r + 1), :, 2:2 + T],
                    start=False,
                    stop=True,
                )

            # maxpool over t -> obuf[:, chunk words]: DVE reduce from PSUM
            red_out = (
                obuf[:, cc * CHUNK_W:(cc + 1) * CHUNK_W]
                .rearrange("f (r w) -> f r w", w=RANGE_W)
            )
            red_in = (
                y_ps[:, :].rearrange("f (r x) -> f r x", x=512)[:, :, 0:UR * 2 * T]
                .rearrange("f r (w t) -> f r w t", t=T)
            )
            nc.vector.tensor_reduce(
                red_out, red_in, axis=mybir.AxisListType.X,
                op=mybir.AluOpType.max,
            )

        # bias + store
        nc.vector.tensor_scalar_add(obuf[:], obuf[:], b_sb[:, 0:1])
        nc.sync.dma_start(out_d.ap(), obuf[:])

    nc.compile()
    return nc


def host_prep(char_ids, emb_table, conv_w, conv_b, num_devices=NCORES):
    """Build per-core input maps from full inputs."""
    char_ids = np.asarray(char_ids)
    emb_table = np.asarray(emb_table, dtype=np.float32)
    conv_w = np.asarray(conv_w, dtype=np.float32)
    conv_b = np.asarray(conv_b, dtype=np.float32)

    bf = mybir.dt.np(bf16)
    tab = emb_table.astype(bf)

    # ids: chunk-local gather column 64*(w//2) + 2*p + (w%2) holds (w, p)
    ids_all = char_ids.reshape(-1, CHUNK_W, C).astype(np.int16)  # [nchunks, w, p]
    nch = ids_all.shape[0]
    ids_perm = (
        ids_all.reshape(nch, CHUNK_W // 2, 2, C)
        .transpose(0, 1, 3, 2)  # [chunk, u, p, eps]
        .reshape(nch, CH_COLS)
    )

    # W layout: q = 32h+p (q<64) -> channel 2p+h
    q = np.arange(64)
    ch = 2 * (q % 32) + q // 32
    wmat = np.zeros((128, 256), dtype=np.float32)
    wmat[0:64, 0:128] = conv_w[:, ch, 0].T
    wmat[64:128, 0:128] = conv_w[:, ch, 1].T
    wmat[0:64, 128:256] = conv_w[:, ch, 2].T
    wmat = wmat.astype(bf)

    bias = conv_b.reshape(128, 1).astype(np.float32)

    chunks_per_core = NCHUNKS
    in_maps = []
    for jcore in range(num_devices):
        ids_core = ids_perm[jcore * chunks_per_core:(jcore + 1) * chunks_per_core]
        in_maps.append(
            {
                "idx": np.ascontiguousarray(ids_core.reshape(1, NCHARS)),
                "tab": tab,
                "wmat": wmat,
                "bias": bias,
            }
        )
    return in_maps


def _ensure_ntff_hook():
    """The agent image's antenv lacks axon_hooks; shim it and install the
    ctypes NTFF profiling hook so trace=True yields HW exec times."""
    import types

    if "antenv.axon_hooks" in sys.modules:
        return
    mod = types.ModuleType("antenv.axon_hooks")
    _hook = [None]
    mod.get_axon_ntff_profile_hook = lambda: _hook[0]
    mod.set_axon_ntff_profile_hook = lambda h: _hook.__setitem__(0, h)
    sys.modules["antenv.axon_hooks"] = mod
    try:
        import antenv

        antenv.axon_hooks = mod
        from trn_agent_boot.trn_boot import _ntff_profile_via_ctypes

        hook = _ntff_profile_via_ctypes("/opt/axon/libaxon_pjrt.so")
        mod.set_axon_ntff_profile_hook(hook)
    except Exception as e:  # degrade to no-trace
        print(f"ntff hook install failed: {e}", file=sys.stderr)


_NC_CACHE = {}


def _get_nc():
    if "nc" not in _NC_CACHE:
        _NC_CACHE["nc"] = build_kernel()
    return _NC_CACHE["nc"]


def kernel(char_ids, emb_table, conv_w, conv_b, trace=False):
    if trace:
        _ensure_ntff_hook()
    nc = _get_nc()
    in_maps = host_prep(char_ids, emb_table, conv_w, conv_b)
    res = run_bass_kernel_spmd(
        nc, in_maps, core_ids=list(range(NCORES)), trace=trace
    )
    # out[f, word] word-linear -> [word, f]
    outs = [res.results[jc]["out"].T for jc in range(NCORES)]
    full = np.concatenate(outs, axis=0).reshape(B, S, F).astype(np.float32)
    if trace:
        return full, res
    return full
